# revision 1
# baseline (speedup 1.0000x reference)
"""Trainium2 Bass kernel for chunked causal linear attention (elu+1 feature map).

Reference computation (B=2, N=2048, D=1024, DHAT=512, H=16, F=32, G=64):
    Q = phi(x @ wq + bq), K = phi(x @ wk + bk), V = x @ wv + bv   (per-head split)
    kv_t = cumsum_t(K_t outer V_t);  Z_t = 1/(Q_t . cumsum_t(K)_t + 1e-6)
    out_t = (Q_t . kv_t) * Z_t;  y = out @ wo + bo
with phi(u) = elu(u) + 1 = min(exp(u), max(u + 1, 1)).

Sharding over 8 cores: core c handles batch b = c//4 and heads 4r..4r+3
(r = c%4).  Each core projects its head slice, runs chunk-parallel linear
attention (chunk C=256: intra-chunk masked QK^T + inter-chunk running state
S[f,g]), and computes a PARTIAL output projection through its 256 rows of
wo.  The host sums the 4 partials per batch plus bo (the output-projection
all-reduce realized at unshard time) — on-device collectives on this setup
cost ~25x their table values, far more than the whole compute.

Matmul operands are bf16 (fp32 PSUM accumulation; fp32 state & denominator
path).  Per chunk s-block, one PE transpose of the combined [V^T|K^T] tile
yields the t-major [V|K|ones] block; the denominator partition-broadcast is
an f32r ones-matmul; phi(u) = min(exp(u), max(u+1, 1)) on ACT+DVE.  A
1-stage software pipeline across (chunk, head) iterations plus per-chunk
interleaved output-projection bursts keeps the PE stream dense.
"""
import os
import sys
import types

sys.path.insert(0, "/opt/trn_rl_repo")

import ml_dtypes
import numpy as np

# ---- problem constants (hardcoded; kernel.py must be self-contained) ----
B, N, D, DHAT, H = 2, 2048, 1024, 512, 16
F = DHAT // H        # 32
G = D // H           # 64
NCORES = 8
CHUNK = 256          # attention chunk along t
NCHUNK = N // CHUNK  # 8
JLOC = 4 * G         # 256 local attention features per core
TB = 512             # projection t-block
BF16NP = ml_dtypes.bfloat16


def _install_ntff_hook():
    """Register the axon NTFF profiling hook (stub antenv lacks axon_hooks)."""
    if "antenv.axon_hooks" in sys.modules:
        return
    try:
        from trn_agent_boot.trn_boot import _ntff_profile_via_ctypes
        hook = _ntff_profile_via_ctypes("/opt/axon/libaxon_pjrt.so")
    except Exception:
        hook = None
    m = types.ModuleType("antenv.axon_hooks")
    m.get_axon_ntff_profile_hook = lambda: hook
    m.set_axon_ntff_profile_hook = lambda h: None
    sys.modules["antenv.axon_hooks"] = m


def build_nc():
    import concourse.bass as bass
    import concourse.mybir as mybir
    import concourse.tile as tile
    from concourse import bacc

    F32 = mybir.dt.float32
    BF16 = mybir.dt.bfloat16
    F32R = mybir.dt.float32r
    AF = mybir.ActivationFunctionType
    ALU = mybir.AluOpType

    nc = bacc.Bacc("TRN2", target_bir_lowering=False, debug=False,
                   num_devices=NCORES)

    KD = D // 128  # 8 contraction tiles

    # ---- per-core DRAM parameters (bf16 operands, pre-tiled on host) ----
    xT_e = nc.declare_dram_parameter("xT", [N // TB, KD, 128, TB],
                                     BF16, isOutput=False)
    wq_e = nc.declare_dram_parameter("wq", [KD, 128, 4 * F], BF16,
                                     isOutput=False)
    wk_e = nc.declare_dram_parameter("wk", [KD, 128, 4 * F], BF16,
                                     isOutput=False)
    wv_e = nc.declare_dram_parameter("wv", [KD * 2, 128, 128], BF16,
                                     isOutput=False)
    wo_e = nc.declare_dram_parameter("wo", [2, 128, D], BF16, isOutput=False)
    bq_e = nc.declare_dram_parameter("bq", [4 * F, 1], F32, isOutput=False)
    bk_e = nc.declare_dram_parameter("bk", [4 * F, 1], F32, isOutput=False)
    bv_e = nc.declare_dram_parameter("bv", [JLOC, 1], F32, isOutput=False)
    y_e = nc.declare_dram_parameter("y", [N // 128, 2, 128, 512], F32,
                                    isOutput=True)

    # causal mask [triu(s0) | ones | triu(s1 vs t-high)] for one 256-chunk:
    # cols 0:256 mask block0 [s0, t 0:256]; cols 256:384 mask block1
    # [s1, t 128:256]
    m0 = np.zeros((128, CHUNK), np.float32)
    tri = np.zeros((128, 128), np.float32)
    for s in range(128):
        m0[s, s:] = 1.0
        tri[s, s:] = 1.0
    maskc_d = nc.inline_tensor(
        np.concatenate([m0, tri], axis=1).astype(BF16NP), "maskc")
    ident_d = nc.inline_tensor(np.eye(128, dtype=np.float32).astype(BF16NP),
                               "identc")
    ones_d = nc.inline_tensor(np.ones((1, 128), np.float32), "onesc")
    # vk_p init: 8 blocks of [128,128], ones at col 96 of each block
    vk0 = np.zeros((128, 8, 128), np.float32)
    vk0[:, :, 96] = 1.0
    vkinit_d = nc.inline_tensor(
        np.ascontiguousarray(vk0.reshape(128, 1024)).astype(BF16NP), "vkinit")

    with tile.TileContext(nc) as tc:
        with (
            tc.tile_pool(name="persist", bufs=1) as pers,
            tc.tile_pool(name="xin", bufs=3) as xin,
            tc.tile_pool(name="ppool", bufs=3, space="PSUM") as pp,
            tc.tile_pool(name="vtpool", bufs=1, space="PSUM") as vtpool,
            tc.tile_pool(name="stpool", bufs=1, space="PSUM") as stpool,
            tc.tile_pool(name="opool", bufs=3, space="PSUM") as opsum,
            tc.tile_pool(name="work", bufs=6) as work,
            tc.tile_pool(name="ssb", bufs=6) as ssb,
            tc.tile_pool(name="sbf16", bufs=5) as sbf16,
            tc.tile_pool(name="ysb", bufs=3) as ysb,
        ):
            # ---- persistent SBUF state; weights load as single DMAs ----
            wq_sb = pers.tile([128, KD * 128], BF16)
            nc.sync.dma_start(
                wq_sb[:].rearrange("p (k c) -> p k c", k=KD),
                wq_e[:].rearrange("k p c -> p k c"))
            wk_sb = pers.tile([128, KD * 128], BF16)
            nc.sync.dma_start(
                wk_sb[:].rearrange("p (k c) -> p k c", k=KD),
                wk_e[:].rearrange("k p c -> p k c"))
            wv_sb = pers.tile([128, KD * 2 * 128], BF16)
            nc.sync.dma_start(
                wv_sb[:].rearrange("p (km c) -> p km c", km=KD * 2),
                wv_e[:].rearrange("km p c -> p km c"))
            wo_sb = pers.tile([128, 2 * D], BF16)

            bq_sb = pers.tile([4 * F, 1], F32)
            bk_sb = pers.tile([4 * F, 1], F32)
            bv_sb = [pers.tile([128, 1], F32, tag=f"bv{i}", name=f"bv{i}")
                     for i in range(2)]
            nc.sync.dma_start(bq_sb[:], bq_e[:])
            nc.sync.dma_start(bk_sb[:], bk_e[:])
            for i in range(2):
                nc.sync.dma_start(bv_sb[i][:], bv_e[128 * i:128 * (i + 1), :])
            bq1_sb = pers.tile([4 * F, 1], F32)
            bk1_sb = pers.tile([4 * F, 1], F32)
            nc.vector.tensor_scalar(bq1_sb[:], bq_sb[:], 1.0, None, op0=ALU.add)
            nc.vector.tensor_scalar(bk1_sb[:], bk_sb[:], 1.0, None, op0=ALU.add)
            eps_sb = pers.tile([1, 1], F32)
            nc.vector.memset(eps_sb[:], 1e-6)

            ones_sb = pers.tile([1, 128], F32R)
            maskc_sb = pers.tile([128, 384], BF16)
            ident = pers.tile([128, 128], BF16)

            # per-head feature-major tiles.  Layout (partition-alignment
            # rules: 64-row reads start at 0/64, 32-row reads at 0/32/64/96):
            #   vkh[h]: rows 0:64 = V^T, rows 64:96 = K^T
            #   qh[h]:  rows 64:96 = Q^T  (base 64 to match K^T for the PE)
            qh = [pers.tile([G + F, N], BF16, tag=f"qh{h}", name=f"qh{h}")
                  for h in range(4)]
            vkh = [pers.tile([G + F, N], BF16, tag=f"vkh{h}", name=f"vkh{h}")
                   for h in range(4)]
            aT = [pers.tile([128, N], BF16, tag=f"aT{i}", name=f"aT{i}")
                  for i in range(2)]
            # t-major [V_t | K_t | ones | zeros] blocks, one [128,128] block
            # per (head, s-block), all in one tile (single init DMA; col 96
            # of each block holds the ones column for the denominator row)
            vk_all = pers.tile([128, 8 * 128], BF16)

            def vk_p(h, s, par=0):
                b = 2 * h + s
                return vk_all[:, 128 * b:128 * (b + 1)]

            # ---- projections, feature-major, per t-block ----
            for tb in range(N // TB):
                tsl = slice(TB * tb, TB * (tb + 1))
                xt = xin.tile([128, KD * TB], BF16, tag="xt", name="xt")
                if tb == 0:
                    # split the first t-block load per k-tile so the first
                    # projection matmul starts after 128KB, not 1MB
                    for k in range(KD):
                        nc.sync.dma_start(xt[:, TB * k:TB * (k + 1)],
                                          xT_e[tb, k])
                else:
                    nc.sync.dma_start(
                        xt[:].rearrange("p (k c) -> p k c", k=KD),
                        xT_e[tb].rearrange("k p c -> p k c"))

                for (w_sb, b_sb, b1_sb, is_q) in (
                        (wq_sb, bq_sb, bq1_sb, True),
                        (wk_sb, bk_sb, bk1_sb, False)):
                    ps = pp.tile([128, TB], F32, tag="proj", name="proj_ps")
                    for k in range(KD):
                        nc.tensor.matmul(ps[:], w_sb[:, 128 * k:128 * (k + 1)],
                                         xt[:, TB * k:TB * (k + 1)],
                                         start=(k == 0), stop=(k == KD - 1))
                    # phi(u) = min(exp(u), max(u + 1, 1)), bias folded in
                    e_sb = work.tile([128, TB], BF16, tag="phi_e", name="phi_e")
                    nc.scalar.activation(e_sb[:], ps[:], AF.Exp, bias=b_sb[:])
                    u_sb = work.tile([128, TB], BF16, tag="phi_u", name="phi_u")
                    nc.vector.tensor_scalar(u_sb[:], ps[:], b1_sb[:], 1.0,
                                            op0=ALU.add, op1=ALU.max)
                    for h in range(4):
                        fr = slice(F * h, F * (h + 1))
                        dst = (qh[h] if is_q else vkh[h])[G:G + F, tsl]
                        nc.vector.tensor_tensor(dst, e_sb[fr, :],
                                                u_sb[fr, :], op=ALU.min)

                for m in range(2):
                    ps = pp.tile([128, TB], F32, tag="proj", name="proj_ps")
                    for k in range(KD):
                        nc.tensor.matmul(
                            ps[:],
                            wv_sb[:, 128 * (2 * k + m):128 * (2 * k + m + 1)],
                            xt[:, TB * k:TB * (k + 1)],
                            start=(k == 0), stop=(k == KD - 1))
                    for half in range(2):
                        h = 2 * m + half
                        gr = slice(G * half, G * (half + 1))
                        nc.scalar.activation(
                            vkh[h][0:G, tsl], ps[gr, :], AF.Identity,
                            bias=bv_sb[m][gr, :])

            # deferred (non-startup-critical) constant loads
            nc.sync.dma_start(
                wo_sb[:].rearrange("p (j c) -> p j c", j=2),
                wo_e[:].rearrange("j p c -> p j c"))
            nc.sync.dma_start(ones_sb[:], ones_d[:].bitcast(F32R))
            nc.sync.dma_start(maskc_sb[:], maskc_d[:])
            nc.sync.dma_start(ident[:], ident_d[:])
            nc.sync.dma_start(vk_all[:], vkinit_d[:])

            # ---- chunked linear attention: 1-stage software pipeline ----
            # stage1(i,h): A matmuls + K/V transposes + mask; stage2(i,h):
            # out/state matmuls + normalize.  stage2 of each (i,h) is emitted
            # after stage1 of the NEXT (i,h) so every engine stream keeps
            # independent work between dependent ops.
            s_prev = [None] * 4    # fp32 running state per head (SBUF)
            s_bf = [None] * 4      # bf16 copy for the PE
            norm_pend = [None]     # stashed even-head normalize work

            def stage1(i, h):
                t0 = CHUNK * i
                csl = slice(t0, t0 + CHUNK)
                ssl0 = slice(t0, t0 + 128)
                ssl1 = slice(t0 + 128, t0 + 256)
                a_ps = pp.tile([128, 384], F32, tag="proj", name="a_ps")
                nc.tensor.matmul(a_ps[:, 0:CHUNK],
                                 vkh[h][G:G + F, ssl0],
                                 qh[h][G:G + F, csl],
                                 start=True, stop=True)
                nc.tensor.matmul(a_ps[:, CHUNK:384],
                                 vkh[h][G:G + F, ssl1],
                                 qh[h][G:G + F, t0 + 128:t0 + 256],
                                 start=True, stop=True)
                vt_ps = vtpool.tile([128, 2 * (F + G)], BF16, tag="vt",
                                    name="vt_ps")
                for sb2, ssl in ((0, ssl0), (1, ssl1)):
                    nc.tensor.transpose(
                        vt_ps[:, (F + G) * sb2:(F + G) * (sb2 + 1)],
                        vkh[h][:, ssl], ident[0:F + G, 0:F + G])
                vk_dst = bass.AP(
                    tensor=vk_all.tensor,
                    offset=vk_all.offset + 256 * h,
                    ap=[vk_all.ap[0], [128, 2], [1, F + G]])
                vt_src = bass.AP(
                    tensor=vt_ps.tensor, offset=vt_ps.offset,
                    ap=[vt_ps.ap[0], [F + G, 2], [1, F + G]])
                nc.vector.tensor_copy(vk_dst, vt_src)
                am_sb = work.tile([128, 384], BF16, tag="am", name="am")
                nc.vector.tensor_tensor(am_sb[:], a_ps[:], maskc_sb[:],
                                        op=ALU.mult)
                return am_sb

            def stage2(i, h, am_sb):
                t0 = CHUNK * i
                csl = slice(t0, t0 + CHUNK)
                atile = aT[h // 2]
                vrow = 64 * (h % 2)
                # out^T: rows 0:64 numerator, row 96 denominator
                o_ps = opsum.tile([128, CHUNK], F32, tag="o", name="o_ps")
                nc.tensor.matmul(o_ps[:], vk_p(h, 0, i % 2)[:], am_sb[:, 0:CHUNK],
                                 start=True, stop=False)
                if s_bf[h] is not None:
                    nc.tensor.matmul(o_ps[:], s_bf[h][G:G + F, :],
                                     qh[h][G:G + F, csl], start=False,
                                     stop=False)
                nc.tensor.matmul(o_ps[:, 128:CHUNK], vk_p(h, 1, i % 2)[:],
                                 am_sb[:, CHUNK:384],
                                 start=False, stop=True)
                # state update S += K_chunk^T [V|K|ones] (fp32 in SBUF)
                if i < NCHUNK - 1:
                    s_ps = stpool.tile([F, 128], F32, tag="st", name="s_ps")
                    nc.tensor.matmul(s_ps[:], vk_p(h, 0, i % 2)[:, G:G + F],
                                     vk_p(h, 0, i % 2)[:],
                                     start=True, stop=False)
                    nc.tensor.matmul(s_ps[:], vk_p(h, 1, i % 2)[:, G:G + F],
                                     vk_p(h, 1, i % 2)[:],
                                     start=False, stop=True)
                    s_sb = ssb.tile([G + F, 128], F32, tag="ssb", name="s_sb")
                    if s_prev[h] is None:
                        nc.vector.tensor_copy(s_sb[G:G + F, :], s_ps[:])
                    else:
                        nc.vector.tensor_tensor(s_sb[G:G + F, :], s_ps[:],
                                                s_prev[h][G:G + F, :],
                                                op=ALU.add)
                    s_prev[h] = s_sb
                    sb16 = sbf16.tile([G + F, 128], BF16, tag="sbf",
                                      name="sb16")
                    nc.vector.tensor_copy(sb16[G:G + F, :], s_sb[G:G + F, :])
                    s_bf[h] = sb16
                # normalize: attn^T = num / (den + 1e-6)  (fp32 path;
                # partition-broadcast of the denominator via f32r ones-matmul)
                den_sb = work.tile([1, CHUNK], F32R, tag="den", name="den")
                nc.scalar.activation(den_sb[:], o_ps[96:97, :],
                                     AF.Identity, bias=eps_sb[:])
                bc_ps = pp.tile([128, CHUNK], F32, tag="proj", name="bc_ps")
                nc.tensor.matmul(bc_ps[:], ones_sb[:], den_sb[:],
                                 start=True, stop=True)
                rec_sb = work.tile([G, CHUNK], F32, tag="rec", name="rec")
                nc.vector.reciprocal_approx_fast(rec_sb[:], bc_ps[0:G, :])
                nc.vector.tensor_tensor(
                    atile[vrow:vrow + G, csl], o_ps[0:G, :], rec_sb[:],
                    op=ALU.mult)

            def yproj(chunks):
                # output projection burst: 2 chunks = 4 t-tiles = 16 matmuls
                # back-to-back (a >3.4us continuous PE stretch re-warms HAM)
                for tt in [t for i in chunks for t in (2 * i, 2 * i + 1)]:
                    tsl = slice(128 * tt, 128 * (tt + 1))
                    for eb in range(2):
                        esl = slice(512 * eb, 512 * (eb + 1))
                        y_ps = opsum.tile([128, 512], F32, tag="o",
                                          name="y_ps")
                        for j in range(2):
                            nc.tensor.matmul(
                                y_ps[:], aT[j][:, tsl],
                                wo_sb[:, D * j:D * j + D][:, esl],
                                start=(j == 0), stop=(j == 1))
                        y_sb = ysb.tile([128, 512], F32, tag="ysb",
                                        name="y_sb")
                        if (tt + eb) % 2 == 0:
                            nc.scalar.activation(y_sb[:], y_ps[:], AF.Copy)
                        else:
                            nc.vector.tensor_copy(y_sb[:], y_ps[:])
                        nc.sync.dma_start(y_e[tt, eb], y_sb[:])


            pend = None
            for i in range(NCHUNK):
                for h in range(4):
                    am = stage1(i, h)
                    if pend is not None:
                        pi, ph, pam = pend
                        stage2(pi, ph, pam)
                        if ph == 3:
                            yproj((pi,))
                    pend = (i, h, am)
            pi, ph, pam = pend
            stage2(pi, ph, pam)
            yproj((pi,))

    nc.compile()
    return nc


def make_in_maps(x, wq, bq, wk, bk, wv, bv, wo, bo):
    x = np.asarray(x, np.float32)
    in_maps = []
    for c in range(NCORES):
        b, r = divmod(c, 4)
        xt_b = x[b].T.astype(BF16NP)                    # (D, N)
        xt_tiled = np.ascontiguousarray(
            xt_b.reshape(D // 128, 128, N // TB, TB).transpose(2, 0, 1, 3))
        wq_s = np.asarray(wq)[:, 128 * r:128 * (r + 1)].astype(BF16NP)
        wk_s = np.asarray(wk)[:, 128 * r:128 * (r + 1)].astype(BF16NP)
        wv_s = np.asarray(wv)[:, 256 * r:256 * (r + 1)].astype(BF16NP)
        wo_s = np.asarray(wo)[256 * r:256 * (r + 1), :].astype(BF16NP)
        in_maps.append({
            "xT": xt_tiled,
            "wq": np.ascontiguousarray(wq_s.reshape(D // 128, 128, 4 * F)),
            "wk": np.ascontiguousarray(wk_s.reshape(D // 128, 128, 4 * F)),
            "wv": np.ascontiguousarray(
                wv_s.reshape(D // 128, 128, 2, 128).transpose(0, 2, 1, 3)
            ).reshape(D // 128 * 2, 128, 128),
            "wo": np.ascontiguousarray(wo_s.reshape(2, 128, D)),
            "bq": np.ascontiguousarray(
                np.asarray(bq)[128 * r:128 * (r + 1)],
                dtype=np.float32).reshape(-1, 1),
            "bk": np.ascontiguousarray(
                np.asarray(bk)[128 * r:128 * (r + 1)],
                dtype=np.float32).reshape(-1, 1),
            "bv": np.ascontiguousarray(
                np.asarray(bv)[256 * r:256 * (r + 1)],
                dtype=np.float32).reshape(-1, 1),
        })
    return in_maps


def assemble(results, bo):
    y = np.zeros((B, N, D), np.float32)
    for c in range(NCORES):
        yt = results[c]["y"]          # (N//128, 2, 128, 512)
        y[c // 4] += yt.transpose(0, 2, 1, 3).reshape(N, D)
    return y + np.asarray(bo, np.float32).reshape(1, 1, D)


_NC_CACHE = {}


def run(inputs, trace=False):
    _install_ntff_hook()
    from concourse.bass_utils import run_bass_kernel_spmd
    if "nc" not in _NC_CACHE:
        _NC_CACHE["nc"] = build_nc()
    nc = _NC_CACHE["nc"]
    in_maps = make_in_maps(**inputs)
    res = run_bass_kernel_spmd(nc, in_maps, core_ids=list(range(NCORES)),
                               trace=trace)
    return assemble(res.results, inputs["bo"]), res.exec_time_ns


def kernel(**inputs) -> np.ndarray:
    y, _ = run(inputs, trace=False)
    return y



# revision 9
# speedup vs baseline: 1.0005x; 1.0005x over previous
"""Trainium2 Bass kernel for chunked causal linear attention (elu+1 feature map).

Reference computation (B=2, N=2048, D=1024, DHAT=512, H=16, F=32, G=64):
    Q = phi(x @ wq + bq), K = phi(x @ wk + bk), V = x @ wv + bv   (per-head split)
    kv_t = cumsum_t(K_t outer V_t);  Z_t = 1/(Q_t . cumsum_t(K)_t + 1e-6)
    out_t = (Q_t . kv_t) * Z_t;  y = out @ wo + bo
with phi(u) = elu(u) + 1 = min(exp(u), max(u + 1, 1)).

Sharding over 8 cores: core c handles batch b = c//4 and heads 4r..4r+3
(r = c%4).  Each core projects its head slice, runs chunk-parallel linear
attention (chunk C=256), and computes a PARTIAL output projection through its
256 rows of wo; the host sums the 4 bf16 partials per batch plus bo.

Layout strategy (v2):
  * Projection phase builds, per 512-t block: feature-major Q^T/K^T (for the
    intra-chunk A matmuls), and t-major [V|K|ones] 128-t blocks in one
    persistent SBUF tile vkT (V projected DIRECTLY t-major with bias folded
    in as a rank-1 matmul; K transposed 4-heads-at-a-time on the PE).
  * Attention is computed t-major: per 128-t block the PE produces
    o[t, 4 x (V,.,den)] so the denominator lands as a per-partition column;
    normalize is then a per-partition-scale ACT op (no PE broadcast, no wide
    DVE reciprocal/multiply).  The normalized attn is transposed back to
    feature-major for the output projection (2 PE transposes / 128-t block).
  * Running state S = cumsum K^T[V|K|ones] stays RESIDENT IN PSUM (PE
    accumulates across chunks); only a [128,128] bf16 cast per chunk runs on
    the DVE.
  * y partials are written bf16 (halves output DMA).
"""
import os
import sys
import types

sys.path.insert(0, "/opt/trn_rl_repo")

import ml_dtypes
import numpy as np

# ---- problem constants (hardcoded; kernel.py must be self-contained) ----
B, N, D, DHAT, H = 2, 2048, 1024, 512, 16
F = DHAT // H        # 32
G = D // H           # 64
NCORES = 8
CHUNK = 256          # attention chunk along t
NCHUNK = N // CHUNK  # 8
SB = 128             # s-block (128-t block)
NSB = N // SB        # 16
TB = 512             # projection t-block
KD = D // 128        # 8 contraction tiles
BF16NP = ml_dtypes.bfloat16


def _install_ntff_hook():
    """Register the axon NTFF profiling hook (stub antenv lacks axon_hooks)."""
    if "antenv.axon_hooks" in sys.modules:
        return
    try:
        from trn_agent_boot.trn_boot import _ntff_profile_via_ctypes
        hook = _ntff_profile_via_ctypes("/opt/axon/libaxon_pjrt.so")
    except Exception:
        hook = None
    m = types.ModuleType("antenv.axon_hooks")
    m.get_axon_ntff_profile_hook = lambda: hook
    m.set_axon_ntff_profile_hook = lambda h: None
    sys.modules["antenv.axon_hooks"] = m


def build_nc():
    import concourse.bass as bass
    import concourse.mybir as mybir
    import concourse.tile as tile
    from concourse import bacc

    F32 = mybir.dt.float32
    BF16 = mybir.dt.bfloat16
    AF = mybir.ActivationFunctionType
    ALU = mybir.AluOpType

    nc = bacc.Bacc("TRN2", target_bir_lowering=False, debug=False,
                   num_devices=NCORES)

    # ---- per-core DRAM parameters (bf16 operands, pre-tiled on host) ----
    xT_e = nc.declare_dram_parameter("xT", [N // TB, KD, 128, TB],
                                     BF16, isOutput=False)
    wq_e = nc.declare_dram_parameter("wq", [KD, 128, 4 * F], BF16,
                                     isOutput=False)
    wk_e = nc.declare_dram_parameter("wk", [KD, 128, 4 * F], BF16,
                                     isOutput=False)
    wv_e = nc.declare_dram_parameter("wv", [KD * 2, 128, 128], BF16,
                                     isOutput=False)
    wo_e = nc.declare_dram_parameter("wo", [2, 128, D], BF16, isOutput=False)
    bq_e = nc.declare_dram_parameter("bq", [4 * F, 1], F32, isOutput=False)
    bk_e = nc.declare_dram_parameter("bk", [4 * F, 1], F32, isOutput=False)
    bv_e = nc.declare_dram_parameter("bv", [1, 4 * G], BF16, isOutput=False)
    y_e = nc.declare_dram_parameter("y", [N // 128, 2, 128, 512], BF16,
                                    isOutput=True)

    # causal mask [triu(s0 vs t) | triu(s1 vs t-high)] for one 256-chunk:
    # cols 0:256 mask block0 [s0, t 0:256]; cols 256:384 mask block1
    # [s1, t 128:256]
    m0 = np.zeros((128, CHUNK), np.float32)
    tri = np.zeros((128, 128), np.float32)
    for s in range(128):
        m0[s, s:] = 1.0
        tri[s, s:] = 1.0
    maskc_d = nc.inline_tensor(
        np.concatenate([m0, tri], axis=1).astype(BF16NP), "maskc")
    ident_d = nc.inline_tensor(np.eye(128, dtype=np.float32).astype(BF16NP),
                               "identc")
    ones1_d = nc.inline_tensor(np.ones((1, 128), np.float32).astype(BF16NP),
                               "ones1c")

    with tile.TileContext(nc) as tc:
        with (
            tc.tile_pool(name="persist", bufs=1) as pers,
            tc.tile_pool(name="xin", bufs=4) as xin,
            tc.tile_pool(name="ppool", bufs=2, space="PSUM") as pp,
            tc.tile_pool(name="avpool", bufs=2, space="PSUM") as av,
            tc.tile_pool(name="opool", bufs=2, space="PSUM") as op,
            tc.tile_pool(name="spool", bufs=1, space="PSUM") as sp,
            tc.tile_pool(name="work", bufs=6) as work,
            tc.tile_pool(name="sbf16", bufs=2) as sbf16p,
            tc.tile_pool(name="ysb", bufs=3) as ysb,
        ):
            # ---- persistent SBUF tiles ----
            wq_sb = pers.tile([128, KD * 128], BF16)
            wk_sb = pers.tile([128, KD * 128], BF16)
            wv_sb = pers.tile([128, KD * 256], BF16)
            wo_sb = pers.tile([128, 2 * D], BF16)
            bq_sb = pers.tile([4 * F, 1], F32)
            bk_sb = pers.tile([4 * F, 1], F32)
            bv_sb = pers.tile([1, 4 * G], BF16)
            ones1 = pers.tile([1, 128], BF16)
            ident = pers.tile([128, 128], BF16)
            maskc_sb = pers.tile([128, 384], BF16)
            qT = pers.tile([128, N], BF16)
            kT = pers.tile([128, N], BF16)
            # t-major [V|K|ones] blocks: s-block j at cols 512j, head h at
            # 512j+128h: [V(0:64) | K(64:96) | ones(96) | zero(97:128)]
            vkT = pers.tile([128, NSB * 512], BF16)
            # feature-major normalized attn: j-half jh (heads 2jh,2jh+1) at
            # cols jh*N + t
            aT = pers.tile([128, 2 * N], BF16)

            # zero vkT, then set the ones columns (col 96 of each block)
            nc.vector.memset(vkT[:], 0.0)
            nc.vector.memset(
                bass.AP(tensor=vkT.tensor, offset=vkT.offset + 96,
                        ap=[vkT.ap[0], [128, 4 * NSB]]), 1.0)

            # ---- startup DMAs: interleave weight/x issue on 2 engines so
            # the first projection matmul starts ~2us in ----
            # sync engine: x t-block 0 (2 halves), ident, x prefetches, wo
            xt_tiles = []
            xt0 = xin.tile([128, KD * TB], BF16, tag="xt", name="xt0")
            nc.sync.dma_start(
                xt0[:, 0:4 * TB].rearrange("p (k c) -> p k c", k=4),
                xT_e[0, 0:4].rearrange("k p c -> p k c"))
            nc.sync.dma_start(
                xt0[:, 4 * TB:].rearrange("p (k c) -> p k c", k=4),
                xT_e[0, 4:8].rearrange("k p c -> p k c"))
            xt_tiles.append(xt0)
            nc.sync.dma_start(ident[:], ident_d[:])
            # scalar engine: weights + biases (issue in first-use order)
            nc.scalar.dma_start(
                wq_sb[:].rearrange("p (k c) -> p k c", k=KD),
                wq_e[:].rearrange("k p c -> p k c"))
            nc.scalar.dma_start(bq_sb[:], bq_e[:])
            nc.scalar.dma_start(bk_sb[:], bk_e[:])
            nc.scalar.dma_start(
                wk_sb[:].rearrange("p (k c) -> p k c", k=KD),
                wk_e[:].rearrange("k p c -> p k c"))
            nc.scalar.dma_start(
                wv_sb[:].rearrange("p (km c) -> p km c", km=KD * 2),
                wv_e[:].rearrange("km p c -> p km c"))
            nc.scalar.dma_start(bv_sb[:], bv_e[:])
            nc.scalar.dma_start(ones1[:], ones1_d[:])

            bq1_sb = pers.tile([4 * F, 1], F32)
            bk1_sb = pers.tile([4 * F, 1], F32)
            nc.vector.tensor_scalar(bq1_sb[:], bq_sb[:], 1.0, None, op0=ALU.add)
            nc.vector.tensor_scalar(bk1_sb[:], bk_sb[:], 1.0, None, op0=ALU.add)

            # remaining prefetches (sync engine), consts needed by chunk 0
            # issued before the later x blocks
            xt1 = xin.tile([128, KD * TB], BF16, tag="xt", name="xt1")
            nc.sync.dma_start(
                xt1[:].rearrange("p (k c) -> p k c", k=KD),
                xT_e[1].rearrange("k p c -> p k c"))
            xt_tiles.append(xt1)
            nc.sync.dma_start(maskc_sb[:], maskc_d[:])
            nc.sync.dma_start(
                wo_sb[:].rearrange("p (j c) -> p j c", j=2),
                wo_e[:].rearrange("j p c -> p j c"))
            for tb in range(2, N // TB):
                xt = xin.tile([128, KD * TB], BF16, tag="xt", name="xt")
                nc.sync.dma_start(
                    xt[:].rearrange("p (k c) -> p k c", k=KD),
                    xT_e[tb].rearrange("k p c -> p k c"))
                xt_tiles.append(xt)

            def vk_blk(jj, h, c0=0, c1=128):
                return bass.AP(
                    tensor=vkT.tensor,
                    offset=vkT.offset + 512 * jj + 128 * h + c0,
                    ap=[vkT.ap[0], [1, c1 - c0]])

            # ---- projection phase, per 512-t block ----
            def proj_tb(tb):
                tsl = slice(TB * tb, TB * (tb + 1))
                xt = xt_tiles[tb]
                for (w_sb, b_sb, b1_sb, dstT) in (
                        (wq_sb, bq_sb, bq1_sb, qT),
                        (wk_sb, bk_sb, bk1_sb, kT)):
                    ps = pp.tile([128, TB], F32, tag="p", name="qk_ps")
                    for k in range(KD):
                        nc.tensor.matmul(ps[:], w_sb[:, 128 * k:128 * (k + 1)],
                                         xt[:, TB * k:TB * (k + 1)],
                                         start=(k == 0), stop=(k == KD - 1))
                    # phi(u) = min(exp(u), max(u + 1, 1)), bias folded in
                    e_sb = work.tile([128, TB], BF16, tag="phi_e", name="phi_e")
                    nc.scalar.activation(e_sb[:], ps[:], AF.Exp, bias=b_sb[:])
                    u_sb = work.tile([128, TB], BF16, tag="phi_u", name="phi_u")
                    nc.vector.tensor_scalar(u_sb[:], ps[:], b1_sb[:], 1.0,
                                            op0=ALU.add, op1=ALU.max)
                    nc.vector.tensor_tensor(dstT[:, tsl], e_sb[:], u_sb[:],
                                            op=ALU.min)
                # t-major V + K-transpose per 128-t s-block
                for j in range(TB // SB):
                    jj = (TB // SB) * tb + j
                    vps = av.tile([128, 256], F32, tag="av", name="v_ps")
                    nc.tensor.matmul(vps[:], ones1[:], bv_sb[:],
                                     start=True, stop=False)
                    for k in range(KD):
                        nc.tensor.matmul(
                            vps[:],
                            xt[:, TB * k + SB * j:TB * k + SB * (j + 1)],
                            wv_sb[:, 256 * k:256 * (k + 1)],
                            start=False, stop=(k == KD - 1))
                    # V psum [128t, 4h x 64g] -> vkT V columns (strided)
                    vdst = bass.AP(tensor=vkT.tensor,
                                   offset=vkT.offset + 512 * jj,
                                   ap=[vkT.ap[0], [128, 4], [1, G]])
                    vsrc = bass.AP(tensor=vps.tensor, offset=vps.offset,
                                   ap=[vps.ap[0], [G, 4], [1, G]])
                    nc.scalar.activation(vdst, vsrc, AF.Copy)
                    # K^T 4-heads one transpose -> vkT K columns (strided)
                    ktp = av.tile([128, 128], BF16, tag="av", name="kt_ps")
                    nc.tensor.transpose(ktp[:], kT[:, SB * jj:SB * (jj + 1)],
                                        ident[:])
                    kdst = bass.AP(tensor=vkT.tensor,
                                   offset=vkT.offset + 512 * jj + G,
                                   ap=[vkT.ap[0], [128, 4], [1, F]])
                    ksrc = bass.AP(tensor=ktp.tensor, offset=ktp.offset,
                                   ap=[ktp.ap[0], [F, 4], [1, F]])
                    nc.vector.tensor_copy(kdst, ksrc)

            # ---- attention, t-major output ----
            s_bf = [None]     # bf16 copy of running state (all 4 heads)
            s_all = sp.tile([128, 128], F32, tag="s", name="s_all")

            def chunk_attn(i):
                t0 = CHUNK * i
                csl = slice(t0, t0 + CHUNK)
                # intra-chunk A^T = K_s . Q_t, masked, per head
                ams = []
                for h in range(4):
                    fsl = slice(32 * h, 32 * (h + 1))
                    tp = (32 * h, 0) if h == 3 else None
                    a_ps = av.tile([128, 384], F32, tag="av", name="a_ps")
                    nc.tensor.matmul(a_ps[:, 0:CHUNK],
                                     kT[fsl, t0:t0 + 128], qT[fsl, csl],
                                     start=True, stop=True, tile_position=tp)
                    nc.tensor.matmul(a_ps[:, CHUNK:384],
                                     kT[fsl, t0 + 128:t0 + 256],
                                     qT[fsl, t0 + 128:t0 + 256],
                                     start=True, stop=True, tile_position=tp)
                    am = work.tile([128, 384], BF16, tag="am", name="am")
                    nc.vector.tensor_tensor(am[:], a_ps[:], maskc_sb[:],
                                            op=ALU.mult)
                    ams.append(am)
                # t-major o per 128-t block: cols 128h+[V(64)|.|den(96)]
                for tt in range(2):
                    jj = 2 * i + tt
                    o_ps = op.tile([128, 512], F32, tag="o", name="o_ps")
                    for h in range(4):
                        fsl = slice(32 * h, 32 * (h + 1))
                        osl = o_ps[:, 128 * h:128 * (h + 1)]
                        if tt == 0:
                            nc.tensor.matmul(osl, ams[h][:, 0:128],
                                             vk_blk(jj, h),
                                             start=True,
                                             stop=(s_bf[0] is None))
                        else:
                            nc.tensor.matmul(osl, ams[h][:, 128:256],
                                             vk_blk(jj - 1, h),
                                             start=True, stop=False)
                            nc.tensor.matmul(osl, ams[h][:, 256:384],
                                             vk_blk(jj, h),
                                             start=False,
                                             stop=(s_bf[0] is None))
                        if s_bf[0] is not None:
                            nc.tensor.matmul(
                                osl, qT[fsl, SB * jj:SB * (jj + 1)],
                                s_bf[0][fsl, :], start=False, stop=True,
                                tile_position=(32 * h, 0) if h == 3 else None)
                    # reciprocal of the 4 denominator columns (96+128h)
                    rec4 = work.tile([128, 4], F32, tag="rec", name="rec4")
                    nc.vector.reciprocal_approx_fast(
                        rec4[:],
                        bass.AP(tensor=o_ps.tensor, offset=o_ps.offset + 96,
                                ap=[o_ps.ap[0], [128, 4]]))
                    # normalize: per-head per-partition scale on ACT
                    at_sb = work.tile([128, 256], BF16, tag="att", name="at_sb")
                    for h in range(4):
                        nc.scalar.activation(
                            at_sb[:, G * h:G * (h + 1)],
                            o_ps[:, 128 * h:128 * h + G],
                            AF.Copy, scale=rec4[:, h:h + 1])
                    # transpose back to feature-major for the out-projection
                    atp = op.tile([128, 256], BF16, tag="at", name="at_ps",
                                  bufs=1)
                    nc.tensor.transpose(atp[:, 0:128], at_sb[:, 0:128],
                                        ident[:])
                    nc.tensor.transpose(atp[:, 128:256], at_sb[:, 128:256],
                                        ident[:])
                    adst = bass.AP(tensor=aT.tensor,
                                   offset=aT.offset + SB * jj,
                                   ap=[aT.ap[0], [N, 2], [1, SB]])
                    asrc = bass.AP(tensor=atp.tensor, offset=atp.offset,
                                   ap=[atp.ap[0], [SB, 2], [1, SB]])
                    nc.vector.tensor_copy(adst, asrc)
                # state S += K_chunk^T [V|K|ones] (PSUM-resident accumulate)
                if i < NCHUNK - 1:
                    for h in range(4):
                        ssl = s_all[32 * h:32 * (h + 1), :]
                        tp = (0, 32 * h) if h == 3 else None
                        nc.tensor.matmul(ssl, vk_blk(2 * i, h, G, G + F),
                                         vk_blk(2 * i, h),
                                         start=(i == 0), stop=False,
                                         skip_group_check=True,
                                         tile_position=tp)
                        nc.tensor.matmul(ssl, vk_blk(2 * i + 1, h, G, G + F),
                                         vk_blk(2 * i + 1, h),
                                         start=False, stop=True,
                                         skip_group_check=True,
                                         tile_position=tp)
                    sb16 = sbf16p.tile([128, 128], BF16, tag="sbf",
                                       name="sb16")
                    nc.vector.tensor_copy(sb16[:], s_all[:])
                    s_bf[0] = sb16

            def yproj(i):
                # output projection for chunk i: 2 t-tiles x 2 e-halves
                for tt in (2 * i, 2 * i + 1):
                    tsl = slice(128 * tt, 128 * (tt + 1))
                    for eb in range(2):
                        esl = slice(512 * eb, 512 * (eb + 1))
                        y_ps = pp.tile([128, 512], F32, tag="p", name="y_ps")
                        for j in range(2):
                            nc.tensor.matmul(
                                y_ps[:], aT[:, N * j:N * (j + 1)][:, tsl],
                                wo_sb[:, D * j:D * j + D][:, esl],
                                start=(j == 0), stop=(j == 1))
                        y_sb = ysb.tile([128, 512], BF16, tag="ysb",
                                        name="y_sb")
                        if (tt + eb) % 2 == 0:
                            nc.scalar.activation(y_sb[:], y_ps[:], AF.Copy)
                        else:
                            nc.vector.tensor_copy(y_sb[:], y_ps[:])
                        nc.sync.dma_start(y_e[tt, eb], y_sb[:])

            # ---- schedule: proj t-blocks, then chunks with yproj deferred
            # one chunk so its PE burst fills the next chunk's DVE phase ----
            proj_tb(0)
            proj_tb(1)
            chunk_attn(0)
            proj_tb(2)
            chunk_attn(1)
            yproj(0)
            proj_tb(3)
            for i in range(2, NCHUNK):
                chunk_attn(i)
                yproj(i - 1)
            yproj(NCHUNK - 1)

    nc.compile()
    return nc


def make_in_maps(x, wq, bq, wk, bk, wv, bv, wo, bo):
    x = np.asarray(x, np.float32)
    in_maps = []
    for c in range(NCORES):
        b, r = divmod(c, 4)
        xt_b = x[b].T.astype(BF16NP)                    # (D, N)
        xt_tiled = np.ascontiguousarray(
            xt_b.reshape(D // 128, 128, N // TB, TB).transpose(2, 0, 1, 3))
        wq_s = np.asarray(wq)[:, 128 * r:128 * (r + 1)].astype(BF16NP)
        wk_s = np.asarray(wk)[:, 128 * r:128 * (r + 1)].astype(BF16NP)
        wv_s = np.asarray(wv)[:, 256 * r:256 * (r + 1)].astype(BF16NP)
        wo_s = np.asarray(wo)[256 * r:256 * (r + 1), :].astype(BF16NP)
        in_maps.append({
            "xT": xt_tiled,
            "wq": np.ascontiguousarray(wq_s.reshape(D // 128, 128, 4 * F)),
            "wk": np.ascontiguousarray(wk_s.reshape(D // 128, 128, 4 * F)),
            "wv": np.ascontiguousarray(
                wv_s.reshape(D // 128, 128, 2, 128).transpose(0, 2, 1, 3)
            ).reshape(D // 128 * 2, 128, 128),
            "wo": np.ascontiguousarray(wo_s.reshape(2, 128, D)),
            "bq": np.ascontiguousarray(
                np.asarray(bq)[128 * r:128 * (r + 1)],
                dtype=np.float32).reshape(-1, 1),
            "bk": np.ascontiguousarray(
                np.asarray(bk)[128 * r:128 * (r + 1)],
                dtype=np.float32).reshape(-1, 1),
            "bv": np.ascontiguousarray(
                np.asarray(bv)[256 * r:256 * (r + 1)].astype(BF16NP)
            ).reshape(1, -1),
        })
    return in_maps


def assemble(results, bo):
    y = np.zeros((B, N, D), np.float32)
    for c in range(NCORES):
        yt = results[c]["y"].astype(np.float32)   # (N//128, 2, 128, 512)
        y[c // 4] += yt.transpose(0, 2, 1, 3).reshape(N, D)
    return y + np.asarray(bo, np.float32).reshape(1, 1, D)


_NC_CACHE = {}


def run(inputs, trace=False):
    _install_ntff_hook()
    from concourse.bass_utils import run_bass_kernel_spmd
    if "nc" not in _NC_CACHE:
        _NC_CACHE["nc"] = build_nc()
    nc = _NC_CACHE["nc"]
    in_maps = make_in_maps(**inputs)
    res = run_bass_kernel_spmd(nc, in_maps, core_ids=list(range(NCORES)),
                               trace=trace)
    return assemble(res.results, inputs["bo"]), res.exec_time_ns


def kernel(**inputs) -> np.ndarray:
    y, _ = run(inputs, trace=False)
    return y
